# revision 1
# baseline (speedup 1.0000x reference)
"""Trainium2 Bass kernel for nn_Block_77369540870380 (dense transformer block).

B=2, T=2048, C=1024, H=16, D=64, DFF=4096, fp32 in/out.

Strategy over 8 NeuronCores:
  - Attention tensor-parallel over heads (2 heads/core); all activations kept in
    "transposed" layout (feature dim on SBUF partitions) so every matmul
    contracts over the partition dim.
  - LayerNorms folded into the adjacent projections: stats via ones-matmuls on
    the PE, the -mu*A correction as an extra K-row inside the projection
    matmuls, rstd applied as a PE-broadcast multiply on the DVE.
  - Per-core head outputs oT are exchanged with a single AllToAll (2MB/rank),
    giving each core all 16 heads for its own 512-token slice; each core then
    runs Wp + residual + LN2 + MLP for its slice only (sequence-split MLP).
  - All large matmuls use float32r (TF32-class, full PE rate at free-dim>=256);
    softmax exp without max-subtraction (scores are small by construction).
"""
import numpy as np

import concourse.bacc as bacc
import concourse.mybir as mybir
import concourse.tile as tile
from concourse.alu_op_type import AluOpType
from concourse.bass_utils import run_bass_kernel_spmd

F32 = mybir.dt.float32
F32R = mybir.dt.float32r
AF = mybir.ActivationFunctionType

B, T, C = 2, 2048, 1024
H, D = 16, 64
DFF = 4 * C
N_CORES = 8
HPC = H // N_CORES            # 2 heads per core
TOK = B * T                   # 4096
TPC = TOK // N_CORES          # 512 own tokens per core
NT = T // 512                 # 4 column-chunks per batch
CT = C // 128                 # 8 feature tiles
GT = DFF // 128               # 32 hidden tiles
EPS = 1e-5
SCALE = float(C) ** -0.5


def _build(has_lnb1: bool, has_b1tot: bool = False):
    nc = bacc.Bacc("TRN2", target_bir_lowering=False, debug=False,
                   num_devices=N_CORES)

    ap = lambda name, shape, kind=None: nc.dram_tensor(name, shape, F32, kind=kind).ap()

    xT = ap("xT", [B, C, T], "ExternalInput")
    wq = ap("wq", [C, 2 * D], "ExternalInput")
    wk = ap("wk", [C, 2 * D], "ExternalInput")
    wv = ap("wv", [C, 2 * D], "ExternalInput")
    exq = ap("exq", [1, 2 * D], "ExternalInput")
    exk = ap("exk", [1, 2 * D], "ExternalInput")
    exv = ap("exv", [1, 2 * D], "ExternalInput")
    cq = ap("cq", [2 * D, 1], "ExternalInput")
    ck = ap("ck", [2 * D, 1], "ExternalInput")
    cv = ap("cv", [2 * D, 1], "ExternalInput")
    wp = ap("wp", [C, C], "ExternalInput")
    bpc = ap("bpc", [128, CT], "ExternalInput")
    w1 = ap("w1", [C, DFF], "ExternalInput")
    b1c = ap("b1c", [128, GT], "ExternalInput")
    w2 = ap("w2", [DFF, C], "ExternalInput")
    b2c = ap("b2c", [128, CT], "ExternalInput")
    xo = ap("xo", [C, TPC], "ExternalInput")
    tri = ap("tri", [128, 128], "ExternalInput")
    idn = ap("idn", [128, 128], "ExternalInput")
    out = ap("out", [C, TPC], "ExternalOutput")

    cc_in = nc.dram_tensor("cc_in", [N_CORES, 2 * D, TPC], F32)
    cc_out = nc.dram_tensor("cc_out", [N_CORES, 2 * D, TPC], F32)

    with tile.TileContext(nc) as tc:
        with tc.tile_pool(name="cst", bufs=1) as cst, \
             tc.tile_pool(name="sb", bufs=2) as sb, \
             tc.tile_pool(name="rows", bufs=1) as rows:

            # ---------- constants ----------
            ones_f = cst.tile([1, 512], F32, tag="ones_f")
            nc.any.memset(ones_f[:], 1.0)
            onesc_f = cst.tile([128, 1], F32, tag="onesc_f")
            nc.any.memset(onesc_f[:], 1.0)
            onesc_r = cst.tile([128, 1], F32R, tag="onesc_r")
            nc.vector.tensor_copy(onesc_r[:], onesc_f[:])
            zf = cst.tile([128, 384], F32, tag="zf")
            nc.any.memset(zf[:], 0.0)
            tri_sb = cst.tile([128, 128], F32, tag="tri")
            nc.sync.dma_start(tri_sb[:], tri)
            idn_r = cst.tile([128, 128], F32R, tag="idn")
            nc.sync.dma_start(idn_r[:], idn.bitcast(F32R))
            bpc_sb = cst.tile([128, CT], F32, tag="bpc")
            nc.sync.dma_start(bpc_sb[:], bpc)
            b1c_sb = cst.tile([128, GT], F32, tag="b1c")
            nc.sync.dma_start(b1c_sb[:], b1c)
            b2c_sb = cst.tile([128, CT], F32, tag="b2c")
            nc.sync.dma_start(b2c_sb[:], b2c)
            ex_t = {}
            for nm, src in (("q", exq), ("k", exk), ("v", exv)):
                t_ = cst.tile([1, 2 * D], F32R, tag=f"ex{nm}")
                nc.sync.dma_start(t_[:], src.bitcast(F32R))
                ex_t[nm] = t_
            c_t = {}
            if has_lnb1:
                for nm, src in (("q", cq), ("k", ck), ("v", cv)):
                    t_ = cst.tile([2 * D, 1], F32, tag=f"c{nm}")
                    nc.sync.dma_start(t_[:], src)
                    c_t[nm] = t_

            with tc.tile_pool(name="wqkv", bufs=1) as wpool, \
                 tc.tile_pool(name="proj", bufs=1) as proj, \
                 tc.tile_pool(name="xp", bufs=1) as xp:
                wq_t, wk_t, wv_t = [], [], []
                for (lst, srcw, tg) in ((wq_t, wq, "wq"), (wk_t, wk, "wk"),
                                        (wv_t, wv, "wv")):
                    big = wpool.tile([128, CT * 2 * D], F32R, tag=tg)
                    nc.sync.dma_start(
                        big[:], srcw.rearrange("(j p) d -> p j d", p=128).bitcast(F32R))
                    for j in range(CT):
                        lst.append(big[:, j * 2 * D:(j + 1) * 2 * D])

                for b in range(B):
                    xt = []
                    for j in range(CT):
                        t_ = xp.tile([128, T], F32R, tag=f"xt{j}")
                        nc.sync.dma_start(
                            t_[:], xT[b, j * 128:(j + 1) * 128, :].bitcast(F32R))
                        xt.append(t_)

                    # ---------- LN1 stats + R broadcast ----------
                    mu_r = rows.tile([1, T], F32R, tag="mu_r")
                    R = proj.tile([128, T], F32, tag="R")
                    with tc.tile_pool(name="psA", bufs=3, space="PSUM") as psA, \
                         tc.tile_pool(name="psA2", bufs=2, space="PSUM") as psA2:
                        for i in range(NT):
                            cols = slice(i * 512, (i + 1) * 512)
                            sp = psA2.tile([1, 512], F32, tag="stat")
                            qp = psA2.tile([1, 512], F32, tag="stat")
                            for j in range(CT):
                                nc.tensor.matmul(sp[:], onesc_r[:], xt[j][:, cols],
                                                 start=(j == 0), stop=(j == CT - 1))
                            for j in range(CT):
                                sq = sb.tile([128, 512], F32R, tag="sqscratch")
                                nc.vector.tensor_tensor(
                                    sq[:], xt[j][:, cols].bitcast(F32),
                                    xt[j][:, cols].bitcast(F32), AluOpType.mult)
                                nc.tensor.matmul(qp[:], onesc_r[:], sq[:],
                                                 start=(j == 0), stop=(j == CT - 1))
                            nc.vector.tensor_scalar_mul(mu_r[:, cols], sp[:], 1.0 / C)
                            rB = sb.tile([1, 512], F32, tag="rowB")
                            nc.vector.tensor_scalar_mul(rB[:], qp[:], 1.0 / C)
                            rC = sb.tile([1, 512], F32, tag="rowC")
                            nc.vector.tensor_tensor(rC[:], mu_r[:, cols].bitcast(F32),
                                                    mu_r[:, cols].bitcast(F32),
                                                    AluOpType.mult)
                            rV = sb.tile([1, 512], F32, tag="rowV")
                            nc.vector.scalar_tensor_tensor(rV[:], rB[:], EPS, rC[:],
                                                           AluOpType.add,
                                                           AluOpType.subtract)
                            rS = sb.tile([1, 512], F32, tag="rowS")
                            nc.scalar.activation(rS[:], rV[:], AF.Sqrt)
                            rI = sb.tile([1, 512], F32, tag="rowI")
                            nc.vector.reciprocal(rI[:], rS[:])
                            nc.gpsimd.partition_broadcast(R[:, cols], rI[:])

                        # ---------- projections ----------
                        qT = kT = vT = None
                        for (wt, nm) in ((wq_t, "q"), (wk_t, "k"), (wv_t, "v")):
                            dst = proj.tile([128, T], F32R, tag=f"{nm}T")
                            for i in range(NT):
                                cols = slice(i * 512, (i + 1) * 512)
                                pp = psA.tile([128, 512], F32, tag="projp")
                                for j in range(CT):
                                    nc.tensor.matmul(pp[:], wt[j], xt[j][:, cols],
                                                     start=(j == 0), stop=False)
                                nc.tensor.matmul(pp[:], ex_t[nm][:], mu_r[0:1, cols],
                                                 start=False, stop=True)
                                nc.vector.tensor_tensor(dst[:, cols], pp[:],
                                                        R[:, cols], AluOpType.mult)
                            if has_lnb1:
                                nc.vector.tensor_scalar_add(dst[:], dst[:].bitcast(F32),
                                                            c_t[nm][:])
                            if nm == "q":
                                qT = dst
                            elif nm == "k":
                                kT = dst
                            else:
                                vT = dst

                    # ---------- v transpose ----------
                    v_sb = []
                    with tc.tile_pool(name="psV", bufs=2, space="PSUM") as psV:
                        for s in range(T // 128):
                            scols = slice(s * 128, (s + 1) * 128)
                            tp = psV.tile([128, 128], F32R, tag="vtp")
                            nc.tensor.transpose(tp[:], vT[:, scols], idn_r[:])
                            vt = proj.tile([128, 2 * (D + 1)], F32R, tag=f"v{s}")
                            nc.vector.tensor_copy(vt[:, 0:D], tp[:, 0:D])
                            nc.vector.tensor_copy(vt[:, D:D + 1], onesc_f[:])
                            nc.vector.tensor_copy(vt[:, D + 1:2 * D + 1], tp[:, D:2 * D])
                            nc.vector.tensor_copy(vt[:, 2 * D + 1:2 * D + 2], onesc_f[:])
                            v_sb.append(vt)

                    # ---------- attention ----------
                    with tc.tile_pool(name="expp", bufs=4) as expp, \
                         tc.tile_pool(name="psS", bufs=4, space="PSUM") as psS, \
                         tc.tile_pool(name="psO", bufs=2, space="PSUM") as psO:
                        for i in range(NT):
                            cols = slice(i * 512, (i + 1) * 512)
                            jmax = 4 * (i + 1)
                            shard = b * NT + i
                            ops = []
                            for h in range(HPC):
                                op_ = psO.tile([D + 1, 512], F32, tag=f"op{h}")
                                ops.append(op_)

                            def sc_exp(j):
                                eh = []
                                scols = slice(j * 128, (j + 1) * 128)
                                dq = j * 128 - i * 512
                                for h in range(HPC):
                                    hp = slice(h * D, (h + 1) * D)
                                    scp = psS.tile([128, 512], F32, tag="scp",
                                                   name=f"scp{h}_{j}")
                                    nc.tensor.matmul(scp[:], kT[hp, scols],
                                                     qT[hp, cols],
                                                     start=True, stop=True)
                                    et = expp.tile([128, 512], F32R, tag=f"e{h}",
                                                   name=f"et{h}_{j}")
                                    if dq > 0:
                                        nc.scalar.activation(et[:, dq:], scp[:, dq:],
                                                             AF.Exp, scale=SCALE)
                                        nc.vector.tensor_copy(et[:, 0:dq],
                                                              zf[:, 0:dq])
                                    else:
                                        nc.scalar.activation(et[:], scp[:], AF.Exp,
                                                             scale=SCALE)
                                    if dq >= 0:
                                        nc.vector.tensor_tensor(
                                            et[:, dq:dq + 128],
                                            et[:, dq:dq + 128].bitcast(F32),
                                            tri_sb[:], AluOpType.mult)
                                    eh.append(et)
                                return eh

                            def o_acc(j, eh):
                                for h in range(HPC):
                                    base = h * (D + 1)
                                    nc.tensor.matmul(ops[h][:],
                                                     v_sb[j][:, base:base + D + 1],
                                                     eh[h][:],
                                                     start=(j == 0),
                                                     stop=(j == jmax - 1))

                            e0 = sc_exp(0)
                            e1 = sc_exp(1)
                            for j in range(2, jmax):
                                e2 = sc_exp(j)
                                o_acc(j - 2, e0)
                                e0, e1 = e1, e2
                            o_acc(jmax - 2, e0)
                            o_acc(jmax - 1, e1)

                            for h in range(HPC):
                                den = sb.tile([1, 512], F32, tag="den")
                                nc.vector.reciprocal(den[:], ops[h][D:D + 1, :])
                                rb_sb = sb.tile([D, 512], F32, tag="rb_sb")
                                nc.gpsimd.partition_broadcast(rb_sb[:], den[:])
                                oc_h = sb.tile([D, 512], F32, tag="och")
                                nc.vector.tensor_tensor(oc_h[:], ops[h][0:D, :],
                                                        rb_sb[:], AluOpType.mult)
                                nc.sync.dma_start(
                                    cc_in.ap()[shard, h * D:(h + 1) * D, :], oc_h[:])

            # ---------- AllToAll: heads x own-tokens exchange ----------
            nc.gpsimd.collective_compute(
                "AllToAll", AluOpType.bypass,
                replica_groups=[list(range(N_CORES))],
                ins=[cc_in.ap().opt()],
                outs=[cc_out.ap().opt()],
            )

            # ---------- own tokens: Wp + residual + LN2 ----------
            with tc.tile_pool(name="mpB", bufs=1) as mpB, \
                 tc.tile_pool(name="psM", bufs=2, space="PSUM") as psM:
                x2 = []
                for m in range(CT):
                    x2m = mpB.tile([128, TPC], F32R, tag=f"x2_{m}")
                    x2.append(x2m)
                with tc.tile_pool(name="mpA", bufs=1) as mpA:
                    wp_t = {}
                    for r in range(N_CORES):
                        big = mpA.tile([128, C], F32R, tag=f"wp{r}")
                        nc.sync.dma_start(
                            big[:], wp[r * 128:(r + 1) * 128, :].bitcast(F32R))
                        for m in range(CT):
                            wp_t[(r, m)] = big[:, m * 128:(m + 1) * 128]
                    xo_big = mpA.tile([128, C // 128 * TPC], F32, tag="xo")
                    nc.sync.dma_start(
                        xo_big[:], xo.rearrange("(m p) t -> p m t", p=128))
                    xo_t = [xo_big[:, m * TPC:(m + 1) * TPC] for m in range(CT)]
                    og = []
                    for r in range(N_CORES):
                        t_ = mpA.tile([128, TPC], F32R, tag=f"og{r}")
                        nc.scalar.dma_start(t_[:], cc_out.ap()[r].bitcast(F32R))
                        og.append(t_)

                    for m in range(CT):
                        pp = psM.tile([128, TPC], F32, tag="attp")
                        for r in range(N_CORES):
                            nc.tensor.matmul(pp[:], wp_t[(r, m)], og[r][:],
                                             start=(r == 0), stop=(r == N_CORES - 1))
                        nc.vector.scalar_tensor_tensor(x2[m][:], pp[:],
                                                       bpc_sb[:, m:m + 1],
                                                       xo_t[m], AluOpType.add,
                                                       AluOpType.add)

                # ---------- LN2 ----------
                sp = psM.tile([1, TPC], F32, tag="stat2")
                qp = psM.tile([1, TPC], F32, tag="stat2")
                for m in range(CT):
                    nc.tensor.matmul(sp[:], onesc_r[:], x2[m][:],
                                     start=(m == 0), stop=(m == CT - 1))
                for m in range(CT):
                    sq = sb.tile([128, TPC], F32R, tag="sqscratch")
                    nc.vector.tensor_tensor(sq[:], x2[m][:].bitcast(F32),
                                            x2[m][:].bitcast(F32), AluOpType.mult)
                    nc.tensor.matmul(qp[:], onesc_r[:], sq[:],
                                     start=(m == 0), stop=(m == CT - 1))
                mu2_r = rows.tile([1, TPC], F32R, tag="mu2_r")
                nc.vector.tensor_scalar_mul(mu2_r[:], sp[:], 1.0 / C)
                rB = sb.tile([1, TPC], F32, tag="rowB")
                nc.vector.tensor_scalar_mul(rB[:], qp[:], 1.0 / C)
                rC = sb.tile([1, TPC], F32, tag="rowC")
                nc.vector.tensor_tensor(rC[:], mu2_r[:].bitcast(F32),
                                        mu2_r[:].bitcast(F32), AluOpType.mult)
                rV = sb.tile([1, TPC], F32, tag="rowV")
                nc.vector.scalar_tensor_tensor(rV[:], rB[:], EPS, rC[:],
                                               AluOpType.add, AluOpType.subtract)
                rS = sb.tile([1, TPC], F32, tag="rowS")
                nc.scalar.activation(rS[:], rV[:], AF.Sqrt)
                rI = sb.tile([1, TPC], F32, tag="rowI")
                nc.vector.reciprocal(rI[:], rS[:])
                R2 = sb.tile([128, TPC], F32, tag="R2")
                nc.gpsimd.partition_broadcast(R2[:], rI[:])
                MB2 = sb.tile([128, TPC], F32, tag="MB2")
                nc.gpsimd.partition_broadcast(MB2[:], mu2_r[:].bitcast(F32))
                for m in range(CT):
                    nc.vector.tensor_tensor(x2[m][:], x2[m][:].bitcast(F32),
                                            MB2[:], AluOpType.subtract)

                # ---------- MLP ----------
                with tc.tile_pool(name="f1p", bufs=1) as f1pool, \
                     tc.tile_pool(name="w1p", bufs=4) as w1p, \
                     tc.tile_pool(name="w2p", bufs=2) as w2p:
                    f1 = []
                    for g in range(GT):
                        w1t = w1p.tile([128, C], F32R, tag="w1t")
                        nc.sync.dma_start(
                            w1t[:], w1[:, g * 128:(g + 1) * 128]
                            .rearrange("(j p) c -> p j c", p=128).bitcast(F32R))
                        pp = psM.tile([128, TPC], F32, tag="f1psum")
                        for j in range(CT):
                            nc.tensor.matmul(pp[:], w1t[:, j * 128:(j + 1) * 128],
                                             x2[j][:],
                                             start=(j == 0), stop=(j == CT - 1))
                        fg = f1pool.tile([128, TPC], F32R, tag=f"f1_{g}")
                        if has_b1tot:
                            ft = sb.tile([128, TPC], F32, tag="f1t")
                            nc.vector.tensor_tensor(ft[:], pp[:], R2[:],
                                                    AluOpType.mult)
                            nc.scalar.activation(fg[:], ft[:], AF.Relu,
                                                 bias=b1c_sb[:, g:g + 1])
                        else:
                            # b1tot == 0 and rstd2 > 0: relu(P*R2) = R2*relu(P);
                            # R2 is applied once on the W2 output instead.
                            nc.scalar.activation(fg[:], pp[:], AF.Relu)
                        f1.append(fg)

                    for m in range(CT):
                        w2t = w2p.tile([128, DFF], F32R, tag="w2t")
                        nc.sync.dma_start(
                            w2t[:], w2[:, m * 128:(m + 1) * 128]
                            .rearrange("(g p) c -> p g c", p=128).bitcast(F32R))
                        pp = psM.tile([128, TPC], F32, tag="f2psum")
                        for g in range(GT):
                            nc.tensor.matmul(pp[:], w2t[:, g * 128:(g + 1) * 128],
                                             f1[g][:],
                                             start=(g == 0), stop=(g == GT - 1))
                        om = sb.tile([128, TPC], F32, tag="om")
                        if has_b1tot:
                            nc.vector.scalar_tensor_tensor(
                                om[:], pp[:], b2c_sb[:, m:m + 1],
                                x2[m][:].bitcast(F32), AluOpType.add, AluOpType.add)
                        else:
                            fr = sb.tile([128, TPC], F32, tag="fr")
                            nc.vector.tensor_tensor(fr[:], pp[:], R2[:],
                                                    AluOpType.mult)
                            nc.vector.scalar_tensor_tensor(
                                om[:], fr[:], b2c_sb[:, m:m + 1],
                                x2[m][:].bitcast(F32), AluOpType.add, AluOpType.add)
                        nc.vector.tensor_tensor(om[:], om[:], MB2[:], AluOpType.add)
                        nc.sync.dma_start(out[m * 128:(m + 1) * 128, :], om[:])

    nc.compile()
    return nc


_CACHE = {}


def _get_nc(has_lnb1: bool, has_b1tot: bool):
    key = (has_lnb1, has_b1tot)
    if key not in _CACHE:
        _CACHE[key] = _build(has_lnb1, has_b1tot)
    return _CACHE[key]


def _prep_inputs(x, Wq, Wk, Wv, Wp, bp, W1, b1, W2, b2,
                 ln1_g, ln1_b, ln2_g, ln2_b):
    f = np.float32
    x = np.asarray(x, f)
    xTf = np.ascontiguousarray(x.transpose(0, 2, 1))          # [B, C, T]
    g1 = np.asarray(ln1_g, f)
    b1l = np.asarray(ln1_b, f)
    g2 = np.asarray(ln2_g, f)
    b2l = np.asarray(ln2_b, f)
    Wq = np.asarray(Wq, f)
    Wk = np.asarray(Wk, f)
    Wv = np.asarray(Wv, f)

    w1g = (np.asarray(W1, f) * g2[:, None])
    b1tot = (b2l.astype(np.float64) @ np.asarray(W1, f).astype(np.float64)
             ).astype(f) + np.asarray(b1, f)

    common = {
        "xT": xTf,
        "wp": np.ascontiguousarray(np.asarray(Wp, f)),
        "bpc": np.ascontiguousarray(np.asarray(bp, f).reshape(CT, 128).T),
        "w1": np.ascontiguousarray(w1g),
        "b1c": np.ascontiguousarray(b1tot.reshape(GT, 128).T),
        "w2": np.ascontiguousarray(np.asarray(W2, f)),
        "b2c": np.ascontiguousarray(np.asarray(b2, f).reshape(CT, 128).T),
        "tri": np.triu(np.ones((128, 128), f)),  # expT[s,t] valid iff t >= s
        "idn": np.eye(128, dtype=f),
    }
    in_maps = []
    for c in range(N_CORES):
        hs = [HPC * c + k for k in range(HPC)]
        m = dict(common)
        for nm, W in (("q", Wq), ("k", Wk), ("v", Wv)):
            Wc_raw = np.concatenate([W[h] for h in hs], axis=1)       # [C, 2D]
            Wc = Wc_raw * g1[:, None]
            m["w" + nm] = np.ascontiguousarray(Wc)
            m["ex" + nm] = np.ascontiguousarray(
                -Wc.sum(axis=0, dtype=np.float64).astype(f)[None, :])
            m["c" + nm] = np.ascontiguousarray(
                (b1l.astype(np.float64) @ Wc_raw.astype(np.float64)
                 ).astype(f)[:, None])
        b_own = c // (N_CORES // B)
        i_own = c % (N_CORES // B)
        m["xo"] = np.ascontiguousarray(
            xTf[b_own][:, i_own * TPC:(i_own + 1) * TPC])
        in_maps.append(m)
    has_lnb1 = bool(np.any(b1l != 0))
    has_b1tot = bool(np.any(b1tot != 0))
    return in_maps, has_lnb1, has_b1tot


def kernel(**inputs):
    in_maps, has_lnb1, has_b1tot = _prep_inputs(**inputs)
    nc = _get_nc(has_lnb1, has_b1tot)
    res = run_bass_kernel_spmd(nc, in_maps, list(range(N_CORES)))
    out = np.empty((B, T, C), np.float32)
    for c in range(N_CORES):
        b_own = c // (N_CORES // B)
        i_own = c % (N_CORES // B)
        out[b_own, i_own * TPC:(i_own + 1) * TPC, :] = res.results[c]["out"].T
    return out



# revision 5
# speedup vs baseline: 1.3044x; 1.3044x over previous
"""Trainium2 Bass kernel for nn_Block_77369540870380 (dense transformer block).

B=2, T=2048, C=1024, H=16, D=64, DFF=4096, fp32 in/out.

Strategy over 8 NeuronCores:
  - Attention tensor-parallel over heads (2 heads/core); all activations kept in
    "transposed" layout (feature dim on SBUF partitions) so every matmul
    contracts over the partition dim.
  - LayerNorms folded into the adjacent projections: stats via ones-matmuls on
    the PE, the -mu*A correction as an extra K-row inside the projection
    matmuls, rstd applied as a PE-broadcast multiply on the DVE.
  - Per-core head outputs oT are exchanged with a single AllToAll (1MB/rank),
    giving each core all 16 heads for its own 512-token slice; each core then
    runs Wp + residual + LN2 + MLP for its slice only (sequence-split MLP).
  - All matmul inputs in bf16 (PSUM accumulation in fp32); softmax exp without
    max-subtraction (scores are small by construction); residual paths fp32.
"""
import numpy as np
import ml_dtypes

import concourse.bacc as bacc
import concourse.mybir as mybir
import concourse.tile as tile
from concourse.alu_op_type import AluOpType
from concourse.bass_utils import run_bass_kernel_spmd

F32 = mybir.dt.float32
BF16 = mybir.dt.bfloat16
AF = mybir.ActivationFunctionType

B, T, C = 2, 2048, 1024
H, D = 16, 64
DFF = 4 * C
N_CORES = 8
HPC = H // N_CORES            # 2 heads per core
TOK = B * T                   # 4096
TPC = TOK // N_CORES          # 512 own tokens per core
NT = T // 512                 # 4 column-chunks per batch
CT = C // 128                 # 8 feature tiles
GT = DFF // 128               # 32 hidden tiles
EPS = 1e-5
SCALE = float(C) ** -0.5


def _build(has_lnb1: bool, has_b1tot: bool = False):
    nc = bacc.Bacc("TRN2", target_bir_lowering=False, debug=False,
                   num_devices=N_CORES)

    def ap(name, shape, dt, kind=None):
        return nc.dram_tensor(name, shape, dt, kind=kind).ap()

    xT = ap("xT", [B, C, T], BF16, "ExternalInput")
    wq = ap("wq", [128, CT * 2 * D], BF16, "ExternalInput")
    wk = ap("wk", [128, CT * 2 * D], BF16, "ExternalInput")
    wv = ap("wv", [128, CT * 2 * D], BF16, "ExternalInput")
    exq = ap("exq", [1, 2 * D], BF16, "ExternalInput")
    exk = ap("exk", [1, 2 * D], BF16, "ExternalInput")
    exv = ap("exv", [1, 2 * D], BF16, "ExternalInput")
    cq = ap("cq", [2 * D, 1], F32, "ExternalInput")
    ck = ap("ck", [2 * D, 1], F32, "ExternalInput")
    cv = ap("cv", [2 * D, 1], F32, "ExternalInput")
    wp = ap("wp", [C, C], BF16, "ExternalInput")
    bpc = ap("bpc", [128, CT], F32, "ExternalInput")
    w1 = ap("w1", [128, GT * C], BF16, "ExternalInput")
    b1c = ap("b1c", [128, GT], F32, "ExternalInput")
    w2 = ap("w2", [128, CT * DFF], BF16, "ExternalInput")
    b2c = ap("b2c", [128, CT], F32, "ExternalInput")
    xo = ap("xo", [C, TPC], F32, "ExternalInput")
    tri = ap("tri", [128, 128], BF16, "ExternalInput")
    idn = ap("idn", [128, 128], BF16, "ExternalInput")
    out = ap("out", [C, TPC], F32, "ExternalOutput")

    cc_in = nc.dram_tensor("cc_in", [N_CORES, 2 * D, TPC], BF16)
    cc_out = nc.dram_tensor("cc_out", [N_CORES, 2 * D, TPC], BF16)

    with tile.TileContext(nc) as tc:
        with tc.tile_pool(name="cst", bufs=1) as cst, \
             tc.tile_pool(name="sb", bufs=2) as sb, \
             tc.tile_pool(name="rows", bufs=1) as rows:

            # ---------- constants ----------
            onesc_f = cst.tile([128, 1], F32, tag="onesc_f")
            nc.any.memset(onesc_f[:], 1.0)
            onesc_b = cst.tile([128, 1], BF16, tag="onesc_b")
            nc.vector.tensor_copy(onesc_b[:], onesc_f[:])
            zf = cst.tile([128, 384], BF16, tag="zf")
            nc.any.memset(zf[:], 0.0)
            tri_sb = cst.tile([128, 128], BF16, tag="tri")
            nc.sync.dma_start(tri_sb[:], tri)
            idn_b = cst.tile([128, 128], BF16, tag="idn")
            nc.sync.dma_start(idn_b[:], idn)
            bpc_sb = cst.tile([128, CT], F32, tag="bpc")
            nc.sync.dma_start(bpc_sb[:], bpc)
            b1c_sb = cst.tile([128, GT], F32, tag="b1c")
            nc.sync.dma_start(b1c_sb[:], b1c)
            b2c_sb = cst.tile([128, CT], F32, tag="b2c")
            nc.sync.dma_start(b2c_sb[:], b2c)
            ex_t = {}
            for nm, src in (("q", exq), ("k", exk), ("v", exv)):
                t_ = cst.tile([1, 2 * D], BF16, tag=f"ex{nm}")
                nc.sync.dma_start(t_[:], src)
                ex_t[nm] = t_
            c_t = {}
            if has_lnb1:
                for nm, src in (("q", cq), ("k", ck), ("v", cv)):
                    t_ = cst.tile([2 * D, 1], F32, tag=f"c{nm}")
                    nc.sync.dma_start(t_[:], src)
                    c_t[nm] = t_

            with tc.tile_pool(name="wqkv", bufs=1) as wpool, \
                 tc.tile_pool(name="proj", bufs=1) as proj, \
                 tc.tile_pool(name="xp", bufs=1) as xp:
                wq_t, wk_t, wv_t = [], [], []
                for (lst, srcw, tg) in ((wq_t, wq, "wq"), (wk_t, wk, "wk"),
                                        (wv_t, wv, "wv")):
                    big = wpool.tile([128, CT * 2 * D], BF16, tag=tg)
                    nc.sync.dma_start(big[:], srcw)
                    for j in range(CT):
                        lst.append(big[:, j * 2 * D:(j + 1) * 2 * D])

                for b in range(B):
                    xt = []
                    for j in range(CT):
                        t_ = xp.tile([128, T], BF16, tag=f"xt{j}")
                        xt.append(t_)
                    # column-split loads (i-chunk order) so the first LN1
                    # stats matmuls unblock after ~1/4 of the x traffic
                    for i in range(NT):
                        cols = slice(i * 512, (i + 1) * 512)
                        for j in range(CT):
                            nc.sync.dma_start(
                                xt[j][:, cols],
                                xT[b, j * 128:(j + 1) * 128, cols])

                    # ---------- LN1 stats + R broadcast ----------
                    mu_r = rows.tile([1, T], BF16, tag="mu_r")
                    R = proj.tile([128, T], F32, tag="R")
                    with tc.tile_pool(name="psA", bufs=3, space="PSUM") as psA, \
                         tc.tile_pool(name="psA2", bufs=2, space="PSUM") as psA2:
                        for i in range(NT):
                            cols = slice(i * 512, (i + 1) * 512)
                            sp = psA2.tile([1, 512], F32, tag="stat")
                            qp = psA2.tile([1, 512], F32, tag="stat")
                            for j in range(CT):
                                nc.tensor.matmul(sp[:], onesc_b[:], xt[j][:, cols],
                                                 start=(j == 0), stop=(j == CT - 1))
                            for j in range(CT):
                                sq = sb.tile([128, 512], BF16, tag="sqscratch")
                                nc.vector.tensor_tensor(
                                    sq[:], xt[j][:, cols], xt[j][:, cols],
                                    AluOpType.mult)
                                nc.tensor.matmul(qp[:], onesc_b[:], sq[:],
                                                 start=(j == 0), stop=(j == CT - 1))
                            nc.vector.tensor_scalar_mul(mu_r[:, cols], sp[:], 1.0 / C)
                            rB = sb.tile([1, 512], F32, tag="rowB")
                            nc.vector.tensor_scalar_mul(rB[:], qp[:], 1.0 / C)
                            rC = sb.tile([1, 512], F32, tag="rowC")
                            nc.vector.tensor_tensor(rC[:], mu_r[:, cols],
                                                    mu_r[:, cols],
                                                    AluOpType.mult)
                            rV = sb.tile([1, 512], F32, tag="rowV")
                            nc.vector.scalar_tensor_tensor(rV[:], rB[:], EPS, rC[:],
                                                           AluOpType.add,
                                                           AluOpType.subtract)
                            rS = sb.tile([1, 512], F32, tag="rowS")
                            nc.scalar.activation(rS[:], rV[:], AF.Sqrt)
                            rI = sb.tile([1, 512], F32, tag="rowI")
                            nc.vector.reciprocal_approx_fast(rI[:], rS[:])
                            nc.gpsimd.partition_broadcast(R[:, cols], rI[:])

                        # ---------- projections ----------
                        qT = kT = vT = None
                        for (wt, nm) in ((wq_t, "q"), (wk_t, "k"), (wv_t, "v")):
                            dst = proj.tile([128, T], BF16, tag=f"{nm}T")
                            for i in range(NT):
                                cols = slice(i * 512, (i + 1) * 512)
                                pp = psA.tile([128, 512], F32, tag="projp")
                                for j in range(CT):
                                    nc.tensor.matmul(pp[:], wt[j], xt[j][:, cols],
                                                     start=(j == 0), stop=False)
                                nc.tensor.matmul(pp[:], ex_t[nm][:], mu_r[0:1, cols],
                                                 start=False, stop=True)
                                nc.vector.tensor_tensor(dst[:, cols], pp[:],
                                                        R[:, cols], AluOpType.mult)
                            if has_lnb1:
                                nc.vector.tensor_scalar_add(dst[:], dst[:],
                                                            c_t[nm][:])
                            if nm == "q":
                                qT = dst
                            elif nm == "k":
                                kT = dst
                            else:
                                vT = dst

                    # ---------- v transpose ----------
                    # per-head stationary block of 96: [ones | 31 zeros | v].
                    # den lands on PSUM partition 0 (required by the custom-DVE
                    # approx reciprocal); o lands on partitions 32..95 (32-
                    # aligned as the BIR verifier requires).
                    v_sb = []
                    with tc.tile_pool(name="psV", bufs=2, space="PSUM") as psV:
                        for s in range(T // 128):
                            scols = slice(s * 128, (s + 1) * 128)
                            tp = psV.tile([128, 128], BF16, tag="vtp")
                            nc.tensor.transpose(tp[:], vT[:, scols], idn_b[:])
                            vt = proj.tile([128, 2 * 128], BF16, tag=f"v{s}")
                            for h in range(HPC):
                                hb = 128 * h
                                nc.vector.tensor_copy(vt[:, hb:hb + 1], onesc_b[:])
                                nc.vector.tensor_copy(vt[:, hb + 1:hb + 64],
                                                      zf[:, 0:63])
                                nc.vector.tensor_copy(vt[:, hb + 64:hb + 128],
                                                      tp[:, h * D:(h + 1) * D])
                            v_sb.append(vt)

                    # ---------- attention ----------
                    with tc.tile_pool(name="expp", bufs=4) as expp, \
                         tc.tile_pool(name="psS", bufs=4, space="PSUM") as psS, \
                         tc.tile_pool(name="psO", bufs=2, space="PSUM") as psO:
                        for i in range(NT):
                            cols = slice(i * 512, (i + 1) * 512)
                            jmax = 4 * (i + 1)
                            shard = b * NT + i
                            ops = []
                            for h in range(HPC):
                                op_ = psO.tile([128, 512], F32, tag=f"op{h}")
                                ops.append(op_)

                            def sc_exp(j):
                                eh = []
                                scols = slice(j * 128, (j + 1) * 128)
                                dq = j * 128 - i * 512
                                for h in range(HPC):
                                    hp = slice(h * D, (h + 1) * D)
                                    scp = psS.tile([128, 512], F32, tag="scp",
                                                   name=f"scp{h}_{j}")
                                    nc.tensor.matmul(scp[:], kT[hp, scols],
                                                     qT[hp, cols],
                                                     start=True, stop=True)
                                    et = expp.tile([128, 512], BF16, tag=f"e{h}",
                                                   name=f"et{h}_{j}")
                                    if dq > 0:
                                        nc.scalar.activation(et[:, dq:], scp[:, dq:],
                                                             AF.Exp, scale=SCALE)
                                        nc.vector.tensor_copy(et[:, 0:dq],
                                                              zf[:, 0:dq])
                                    else:
                                        nc.scalar.activation(et[:], scp[:], AF.Exp,
                                                             scale=SCALE)
                                    if dq >= 0:
                                        nc.vector.tensor_tensor(
                                            et[:, dq:dq + 128],
                                            et[:, dq:dq + 128],
                                            tri_sb[:], AluOpType.mult)
                                    eh.append(et)
                                return eh

                            def o_acc(j, eh):
                                for h in range(HPC):
                                    base = h * 128
                                    nc.tensor.matmul(ops[h][:],
                                                     v_sb[j][:, base:base + 128],
                                                     eh[h][:],
                                                     start=(j == 0),
                                                     stop=(j == jmax - 1))

                            e0 = sc_exp(0)
                            e1 = sc_exp(1)
                            for j in range(2, jmax):
                                e2 = sc_exp(j)
                                o_acc(j - 2, e0)
                                e0, e1 = e1, e2
                            o_acc(jmax - 2, e0)
                            o_acc(jmax - 1, e1)

                            for h in range(HPC):
                                den = sb.tile([1, 512], F32, tag="den")
                                nc.vector.reciprocal_approx_fast(
                                    den[:], ops[h][0:1, :])
                                rb_sb = sb.tile([D, 512], F32, tag="rb_sb")
                                nc.gpsimd.partition_broadcast(rb_sb[:], den[:])
                                oc_h = sb.tile([D, 512], BF16, tag="och")
                                nc.vector.tensor_tensor(oc_h[:], ops[h][64:128, :],
                                                        rb_sb[:], AluOpType.mult)
                                nc.sync.dma_start(
                                    cc_in.ap()[shard, h * D:(h + 1) * D, :], oc_h[:])

            # ---------- AllToAll: heads x own-tokens exchange ----------
            nc.gpsimd.collective_compute(
                "AllToAll", AluOpType.bypass,
                replica_groups=[list(range(N_CORES))],
                ins=[cc_in.ap().opt()],
                outs=[cc_out.ap().opt()],
            )

            # ---------- own tokens: Wp + residual + LN2 ----------
            with tc.tile_pool(name="mpB", bufs=1) as mpB, \
                 tc.tile_pool(name="psM", bufs=2, space="PSUM") as psM:
                x2 = []
                for m in range(CT):
                    x2m = mpB.tile([128, TPC], F32, tag=f"x2_{m}")
                    x2.append(x2m)
                with tc.tile_pool(name="mpA", bufs=1) as mpA:
                    wp_t = {}
                    for r in range(N_CORES):
                        big = mpA.tile([128, C], BF16, tag=f"wp{r}")
                        nc.sync.dma_start(big[:], wp[r * 128:(r + 1) * 128, :])
                        for m in range(CT):
                            wp_t[(r, m)] = big[:, m * 128:(m + 1) * 128]
                    xo_big = mpA.tile([128, C // 128 * TPC], F32, tag="xo")
                    nc.sync.dma_start(
                        xo_big[:], xo.rearrange("(m p) t -> p m t", p=128))
                    xo_t = [xo_big[:, m * TPC:(m + 1) * TPC] for m in range(CT)]
                    og = []
                    for r in range(N_CORES):
                        t_ = mpA.tile([128, TPC], BF16, tag=f"og{r}")
                        nc.scalar.dma_start(t_[:], cc_out.ap()[r])
                        og.append(t_)

                    for m in range(CT):
                        pp = psM.tile([128, TPC], F32, tag="attp")
                        for r in range(N_CORES):
                            nc.tensor.matmul(pp[:], wp_t[(r, m)], og[r][:],
                                             start=(r == 0), stop=(r == N_CORES - 1))
                        nc.vector.scalar_tensor_tensor(x2[m][:], pp[:],
                                                       bpc_sb[:, m:m + 1],
                                                       xo_t[m], AluOpType.add,
                                                       AluOpType.add)

                # ---------- LN2 ----------
                x2c = []
                for m in range(CT):
                    t_ = sb.tile([128, TPC], BF16, tag=f"x2c{m}")
                    nc.vector.tensor_copy(t_[:], x2[m][:])
                    x2c.append(t_)
                sp = psM.tile([1, TPC], F32, tag="stat2")
                qp = psM.tile([1, TPC], F32, tag="stat2")
                for m in range(CT):
                    nc.tensor.matmul(sp[:], onesc_b[:], x2c[m][:],
                                     start=(m == 0), stop=(m == CT - 1))
                for m in range(CT):
                    sq = sb.tile([128, TPC], BF16, tag="sqscratch")
                    nc.vector.tensor_tensor(sq[:], x2c[m][:], x2c[m][:],
                                            AluOpType.mult)
                    nc.tensor.matmul(qp[:], onesc_b[:], sq[:],
                                     start=(m == 0), stop=(m == CT - 1))
                mu2_r = rows.tile([1, TPC], F32, tag="mu2_r")
                nc.vector.tensor_scalar_mul(mu2_r[:], sp[:], 1.0 / C)
                rB = sb.tile([1, TPC], F32, tag="rowB")
                nc.vector.tensor_scalar_mul(rB[:], qp[:], 1.0 / C)
                rC = sb.tile([1, TPC], F32, tag="rowC")
                nc.vector.tensor_tensor(rC[:], mu2_r[:], mu2_r[:],
                                        AluOpType.mult)
                rV = sb.tile([1, TPC], F32, tag="rowV")
                nc.vector.scalar_tensor_tensor(rV[:], rB[:], EPS, rC[:],
                                               AluOpType.add, AluOpType.subtract)
                rS = sb.tile([1, TPC], F32, tag="rowS")
                nc.scalar.activation(rS[:], rV[:], AF.Sqrt)
                rI = sb.tile([1, TPC], F32, tag="rowI")
                nc.vector.reciprocal_approx_fast(rI[:], rS[:])
                R2 = sb.tile([128, TPC], F32, tag="R2")
                nc.gpsimd.partition_broadcast(R2[:], rI[:])
                MB2 = sb.tile([128, TPC], F32, tag="MB2")
                nc.gpsimd.partition_broadcast(MB2[:], mu2_r[:])
                x2b = []
                for m in range(CT):
                    t_ = sb.tile([128, TPC], BF16, tag=f"x2b{m}")
                    nc.vector.tensor_tensor(t_[:], x2[m][:], MB2[:],
                                            AluOpType.subtract)
                    x2b.append(t_)

                # ---------- MLP ----------
                with tc.tile_pool(name="f1p", bufs=1) as f1pool, \
                     tc.tile_pool(name="w1p", bufs=4) as w1p, \
                     tc.tile_pool(name="w2p", bufs=2) as w2p:
                    f1 = []
                    for g in range(GT):
                        w1t = w1p.tile([128, C], BF16, tag="w1t")
                        nc.sync.dma_start(w1t[:], w1[:, g * C:(g + 1) * C])
                        pp = psM.tile([128, TPC], F32, tag="f1psum")
                        for j in range(CT):
                            nc.tensor.matmul(pp[:], w1t[:, j * 128:(j + 1) * 128],
                                             x2b[j][:],
                                             start=(j == 0), stop=(j == CT - 1))
                        fg = f1pool.tile([128, TPC], BF16, tag=f"f1_{g}")
                        if has_b1tot:
                            ft = sb.tile([128, TPC], F32, tag="f1t")
                            nc.vector.tensor_tensor(ft[:], pp[:], R2[:],
                                                    AluOpType.mult)
                            nc.scalar.activation(fg[:], ft[:], AF.Relu,
                                                 bias=b1c_sb[:, g:g + 1])
                        else:
                            # b1tot == 0 and rstd2 > 0: relu(P*R2) = R2*relu(P);
                            # R2 is applied once on the W2 output instead.
                            nc.scalar.activation(fg[:], pp[:], AF.Relu)
                        f1.append(fg)

                    for m in range(CT):
                        w2t = w2p.tile([128, DFF], BF16, tag="w2t")
                        nc.sync.dma_start(w2t[:], w2[:, m * DFF:(m + 1) * DFF])
                        pp = psM.tile([128, TPC], F32, tag="f2psum")
                        for g in range(GT):
                            nc.tensor.matmul(pp[:], w2t[:, g * 128:(g + 1) * 128],
                                             f1[g][:],
                                             start=(g == 0), stop=(g == GT - 1))
                        om = sb.tile([128, TPC], F32, tag="om")
                        if has_b1tot:
                            nc.vector.scalar_tensor_tensor(
                                om[:], pp[:], b2c_sb[:, m:m + 1],
                                x2[m][:], AluOpType.add, AluOpType.add)
                        else:
                            fr = sb.tile([128, TPC], F32, tag="fr")
                            nc.vector.tensor_tensor(fr[:], pp[:], R2[:],
                                                    AluOpType.mult)
                            nc.vector.scalar_tensor_tensor(
                                om[:], fr[:], b2c_sb[:, m:m + 1],
                                x2[m][:], AluOpType.add, AluOpType.add)
                        nc.vector.tensor_tensor(om[:], om[:], MB2[:], AluOpType.add)
                        nc.sync.dma_start(out[m * 128:(m + 1) * 128, :], om[:])

    nc.compile()
    return nc


_CACHE = {}


def _get_nc(has_lnb1: bool, has_b1tot: bool):
    key = (has_lnb1, has_b1tot)
    if key not in _CACHE:
        _CACHE[key] = _build(has_lnb1, has_b1tot)
    return _CACHE[key]


def _bf(a):
    return np.ascontiguousarray(np.asarray(a, np.float32).astype(ml_dtypes.bfloat16))


def _prep_inputs(x, Wq, Wk, Wv, Wp, bp, W1, b1, W2, b2,
                 ln1_g, ln1_b, ln2_g, ln2_b):
    f = np.float32
    x = np.asarray(x, f)
    xTf = np.ascontiguousarray(x.transpose(0, 2, 1))          # [B, C, T]
    g1 = np.asarray(ln1_g, f)
    b1l = np.asarray(ln1_b, f)
    g2 = np.asarray(ln2_g, f)
    b2l = np.asarray(ln2_b, f)
    Wq = np.asarray(Wq, f)
    Wk = np.asarray(Wk, f)
    Wv = np.asarray(Wv, f)

    w1g = (np.asarray(W1, f) * g2[:, None])
    b1tot = (b2l.astype(np.float64) @ np.asarray(W1, f).astype(np.float64)
             ).astype(f) + np.asarray(b1, f)
    W2f = np.asarray(W2, f)

    # host pre-arranged contiguous layouts (partition dim first)
    w1r = w1g.reshape(CT, 128, GT, 128).transpose(1, 2, 0, 3).reshape(128, GT * C)
    w2r = W2f.reshape(GT, 128, CT, 128).transpose(1, 2, 0, 3).reshape(128, CT * DFF)

    common = {
        "xT": _bf(xTf),
        "wp": _bf(Wp),
        "bpc": np.ascontiguousarray(np.asarray(bp, f).reshape(CT, 128).T),
        "w1": _bf(w1r),
        "b1c": np.ascontiguousarray(b1tot.reshape(GT, 128).T),
        "w2": _bf(w2r),
        "b2c": np.ascontiguousarray(np.asarray(b2, f).reshape(CT, 128).T),
        "tri": _bf(np.triu(np.ones((128, 128), f))),  # expT[s,t] valid iff t >= s
        "idn": _bf(np.eye(128, dtype=f)),
    }
    in_maps = []
    for c in range(N_CORES):
        hs = [HPC * c + k for k in range(HPC)]
        m = dict(common)
        for nm, W in (("q", Wq), ("k", Wk), ("v", Wv)):
            Wc_raw = np.concatenate([W[h] for h in hs], axis=1)       # [C, 2D]
            Wc = Wc_raw * g1[:, None]
            m["w" + nm] = _bf(
                Wc.reshape(CT, 128, 2 * D).transpose(1, 0, 2).reshape(128, -1))
            m["ex" + nm] = _bf(
                -Wc.sum(axis=0, dtype=np.float64).astype(f)[None, :])
            m["c" + nm] = np.ascontiguousarray(
                (b1l.astype(np.float64) @ Wc_raw.astype(np.float64)
                 ).astype(f)[:, None])
        b_own = c // (N_CORES // B)
        i_own = c % (N_CORES // B)
        m["xo"] = np.ascontiguousarray(
            xTf[b_own][:, i_own * TPC:(i_own + 1) * TPC])
        in_maps.append(m)
    has_lnb1 = bool(np.any(b1l != 0))
    has_b1tot = bool(np.any(b1tot != 0))
    return in_maps, has_lnb1, has_b1tot


def kernel(**inputs):
    in_maps, has_lnb1, has_b1tot = _prep_inputs(**inputs)
    nc = _get_nc(has_lnb1, has_b1tot)
    res = run_bass_kernel_spmd(nc, in_maps, list(range(N_CORES)))
    out = np.empty((B, T, C), np.float32)
    for c in range(N_CORES):
        b_own = c // (N_CORES // B)
        i_own = c % (N_CORES // B)
        out[b_own, i_own * TPC:(i_own + 1) * TPC, :] = res.results[c]["out"].T
    return out
